# revision 2
# baseline (speedup 1.0000x reference)
"""Multi-head attention (B=2, S=2048, E=1024, H=16) on 8 TRN2 NeuronCores.

Sharding: batch x head-group. Core c handles batch b=c//4 and head group
g=c%4 (4 heads = 256 of E). Each core computes its heads' attention output
slice and a partial fc_out product [S, E]; the host sums the 4 partials per
batch and adds b_out.

Device-side math per core (all matmuls in float32r, full PE rate):
  qpT = (Wq_g @ q[b].T + bq)      [256, S]   (T layout: dims on partitions)
  kpT = (Wk_g @ k_c[b].T + bk)    [256, SKV] (k compressed by mask, padded)
  vp  = (v_c[b] @ Wv_g.T + bv)*m  [SKV, 4*65] (per head: 64 dims + ones col)
  S_T = kpT_h.T-chunks @ qpT_h    [SKV, S] per head (2 heads row-packed, K=64)
  E_T = exp(S_T)                  (no max-subtraction: |energy| <~ 60, safe)
  AV  = vp_aug.T @ E_T  -> [65, S]: rows 0-63 = unnormalized O_T, row 64 =
        softmax denominator (ones-column trick; pad rows contribute 0)
  O_T = AV[0:64] / AV[64]         (recip + gpsimd partition-broadcast)
  out_partial = O_T.T @ Wo_g.T    [S, E]

Mask handling is exact: masked K/V rows are removed on the host (gather),
so softmax(where(mask==0, -1e20, e)) == exp(e_valid)/sum(exp(e_valid)).
"""

import os

import numpy as np

B, S, E, H = 2, 2048, 1024, 16
D = E // H           # 64
NCORES = 8
GROUPS = 4           # head groups per batch (cores per batch)
HPG = H // GROUPS    # 4 heads per core
DC = E // GROUPS     # 256 dims per core
NB = E // 128        # 8 contraction chunks over E
SQB = 256            # sq block width for scores/AV
NSQB = S // SQB      # 8

_CACHE = {}


def _split_excess_waits(nc, max_waits=1):
    """walrus rejects instructions carrying >1 sem wait; spread extras onto
    single-wait NoOps inserted before the instruction on the same engine."""
    import concourse.mybir as mybir

    n_split = 0
    for f in nc.m.functions:
        for bb in f.blocks:
            out, changed = [], False
            for ins in bb.instructions:
                si = ins.sync_info
                if si is not None and si.on_wait is not None and len(si.on_wait) > max_waits:
                    waits = list(si.on_wait)
                    for w in waits[:-max_waits]:
                        out.append(mybir.InstNoOp(
                            name=nc.get_next_instruction_name(),
                            engine=ins.engine, ins=[], outs=[],
                            sync_info=mybir.SyncInfo(on_wait=[w], on_update=[])))
                        n_split += 1
                    ins.sync_info = mybir.SyncInfo(
                        on_wait=waits[-max_waits:], on_update=list(si.on_update))
                    changed = True
                out.append(ins)
            if changed:
                bb.instructions = out
    return n_split


def _build(skv, split_waits=True):
    import concourse.bass as bass
    import concourse.mybir as mybir
    import concourse.tile as tile

    f32 = mybir.dt.float32
    f32r = mybir.dt.float32r
    f16 = mybir.dt.float16
    bf16 = mybir.dt.bfloat16
    Alu = mybir.AluOpType
    Act = mybir.ActivationFunctionType

    nsk = skv // 128
    kblocks = []
    rem = skv
    while rem > 0:
        w = 384 if rem % 384 == 0 else min(256, rem)
        kblocks.append(w)
        rem -= w

    nc = bass.Bass()
    xqT = nc.declare_dram_parameter("xqT", [E, S], f32r, isOutput=False)
    xkT = nc.declare_dram_parameter("xkT", [E, skv], f32r, isOutput=False)
    xvT = nc.declare_dram_parameter("xvT", [E, skv], f16, isOutput=False)
    wqT = nc.declare_dram_parameter("wqT", [E, DC], f32r, isOutput=False)
    wkT = nc.declare_dram_parameter("wkT", [E, DC], f32r, isOutput=False)
    wvT = nc.declare_dram_parameter("wvT", [E, DC], f16, isOutput=False)
    woT = nc.declare_dram_parameter("woT", [DC, E], f16, isOutput=False)
    bq_d = nc.declare_dram_parameter("bq", [DC], f32, isOutput=False)
    bk_d = nc.declare_dram_parameter("bk", [DC], f32, isOutput=False)
    bv_d = nc.declare_dram_parameter("bv", [DC], f32, isOutput=False)
    vm_d = nc.declare_dram_parameter("vmask", [skv], f32, isOutput=False)
    ones_d = nc.declare_dram_parameter("ones64", [1, 64], f32r, isOutput=False)
    out_d = nc.declare_dram_parameter("out", [2, S, E], f16, isOutput=True)
    srow_d = nc.dram_tensor("srow", [2, 2, S], f32)
    rrow_d = nc.dram_tensor("rrow", [2, 2, S], f32r)

    xqT_r = xqT.rearrange("(ko p) s -> p ko s", p=128)
    xkT_r = xkT.rearrange("(ko p) s -> p ko s", p=128)
    xvT_r = xvT.rearrange("(ko p) s -> p ko s", p=128)

    QB = 512

    with tile.TileContext(nc) as tc:
        with (
            tc.tile_pool(name="weights", bufs=4) as wpool,
            tc.tile_pool(name="consts", bufs=1) as cpool,
            tc.tile_pool(name="persist", bufs=1) as ppool,
            tc.tile_pool(name="small", bufs=2) as smpool,
            tc.tile_pool(name="proj_ps", bufs=1, space="PSUM") as pps,
            tc.tile_pool(name="stream", bufs=2) as spool,
            tc.tile_pool(name="att_ps", bufs=2, space="PSUM") as aps,
            tc.tile_pool(name="av_ps", bufs=1, space="PSUM") as avps,
            tc.tile_pool(name="fc_ps", bufs=2, space="PSUM") as fps,
            tc.tile_pool(name="et", bufs=3) as etpool,
            tc.tile_pool(name="outp", bufs=3) as opool,
            tc.tile_pool(name="sums", bufs=2) as supool,
            tc.tile_pool(name="rcr", bufs=1) as rcpool,
        ):
            # ---- weights / constants (k first: kpT gates attention) ----
            wk_t = wpool.tile([128, NB, DC], f32r, tag="w", name="wk_t")
            wq_t = wpool.tile([128, NB, DC], f32r, tag="w", name="wq_t")
            wv_t = wpool.tile([128, NB, DC], f16, tag="w", name="wv_t")
            wo_t = wpool.tile([128, DC // 128, E], f16, tag="w", name="wo_t")
            nc.sync.dma_start(wk_t[:], wkT.rearrange("(ko p) m -> p ko m", p=128))
            bq_t = cpool.tile([128, 2], f32, tag="bq")
            bk_t = cpool.tile([128, 2], f32, tag="bk")
            bv_t = cpool.tile([128, DC], f32, tag="bv")
            vm_t = cpool.tile([128, nsk], f32, tag="vm")
            nc.sync.dma_start(bk_t[:], bk_d.rearrange("(c p) -> p c", p=128))
            nc.sync.dma_start(bq_t[:], bq_d.rearrange("(c p) -> p c", p=128))
            nc.sync.dma_start(bv_t[:], bv_d[None, :].to_broadcast((128, DC)))
            nc.sync.dma_start(vm_t[:], vm_d.rearrange("(s p) -> p s", p=128))
            ones_t = cpool.tile([1, 64], f32r, tag="ones")
            nc.sync.dma_start(ones_t[:], ones_d[:])

            qpT = ppool.tile([128, 2, S], f32r, tag="qpT")
            kpT = ppool.tile([128, 2, skv], f32r, tag="kpT")
            vp = ppool.tile([128, nsk, HPG * (D + 1)], bf16, tag="vp")
            o_un = ppool.tile([128, 2, S], f32, tag="o_un")
            o_f16 = ppool.tile([128, 2, S], f16, tag="o_f16")

            def proj_k():
                off = 0
                for w in kblocks:
                    xk = spool.tile([128, NB, max(kblocks)], f32r, tag="xk", name="xk")
                    nc.sync.dma_start(xk[:, :, :w], xkT_r[:, :, off:off + w])
                    for mc in range(2):
                        ps = pps.tile([128, 512], f32, tag="pp", name="kp_ps")[:, :max(kblocks)]
                        for kc in range(NB):
                            nc.tensor.matmul(
                                ps[:, :w], wk_t[:, kc, mc * 128:(mc + 1) * 128],
                                xk[:, kc, :w], start=(kc == 0), stop=(kc == NB - 1))
                        nc.vector.tensor_tensor(
                            out=kpT[:, mc, off:off + w], in0=ps[:, :w],
                            in1=bk_t[:, mc:mc + 1].to_broadcast((128, w)), op=Alu.add)
                    off += w

            def proj_q(nb):
                xq = spool.tile([128, NB, 512], f32r, tag="xq", name="xq")
                nc.sync.dma_start(xq[:], xqT_r[:, :, nb * 512:(nb + 1) * 512])
                for mc in range(2):
                    ps = pps.tile([128, 512], f32, tag="pp", name="qp_ps")
                    for kc in range(NB):
                        nc.tensor.matmul(
                            ps[:], wq_t[:, kc, mc * 128:(mc + 1) * 128],
                            xq[:, kc, :], start=(kc == 0), stop=(kc == NB - 1))
                    nc.vector.tensor_tensor(
                        out=qpT[:, mc, nb * 512:(nb + 1) * 512], in0=ps[:],
                        in1=bq_t[:, mc:mc + 1].to_broadcast((128, 512)), op=Alu.add)

            def proj_v(sc):
                xv = spool.tile([128, NB, 128], f16, tag="xv", name="xv")
                nc.sync.dma_start(xv[:], xvT_r[:, :, sc * 128:(sc + 1) * 128])
                ps = pps.tile([128, 512], f32, tag="pp", name="vp_ps")[:, :DC]
                for kc in range(NB):
                    nc.tensor.matmul(
                        ps[:], xv[:, kc, :], wv_t[:, kc, :],
                        start=(kc == 0), stop=(kc == NB - 1))
                t1 = smpool.tile([128, DC], f32, tag="vtmp")
                nc.vector.tensor_tensor(out=t1[:], in0=ps[:], in1=bv_t[:], op=Alu.add)
                vps = vp[:, sc, :].rearrange("p (h w) -> p h w", w=D + 1)
                nc.vector.tensor_tensor(
                    out=vps[:, :, 0:D],
                    in0=t1.rearrange("p (h w) -> p h w", w=D),
                    in1=vm_t[:, sc:sc + 1, None].to_broadcast((128, HPG, D)),
                    op=Alu.mult)
                nc.vector.tensor_copy(
                    out=vps[:, :, D:D + 1],
                    in_=vm_t[:, sc:sc + 1, None].to_broadcast((128, HPG, 1)))

            # lead-in: kpT fully, first qpT block, vp
            proj_k()
            nc.sync.dma_start(wq_t[:], wqT.rearrange("(ko p) m -> p ko m", p=128))
            proj_q(0)
            nc.sync.dma_start(wv_t[:], wvT.rearrange("(ko p) m -> p ko m", p=128))
            for sc in range(nsk):
                proj_v(sc)
            nc.sync.dma_start(wo_t[:], woT.rearrange("(ko p) n -> p ko n", p=128))

            for pt in range(2):
                sums = [supool.tile([1, S], f32, tag="sums", name=f"sums{j}")
                        for j in range(2)]
                for qb in range(S // QB):
                    q0 = qb * QB
                    et = [etpool.tile([128, nsk, QB], bf16, tag="et", name=f"et{j}")
                          for j in range(2)]
                    for skc in range(nsk):
                        psx = [aps.tile([128, QB], f32, tag=f"sc{j}", name=f"psx{j}")
                               for j in range(2)]
                        for j in range(2):
                            nc.tensor.matmul(
                                psx[j][:],
                                kpT[64 * j:64 * j + 64, pt, skc * 128:(skc + 1) * 128],
                                qpT[64 * j:64 * j + 64, pt, q0:q0 + QB],
                                start=True, stop=True, tile_position=(64 * j, 0))
                        for j in range(2):
                            nc.scalar.activation(et[j][:, skc, :], psx[j][:], Act.Exp)
                    for j in range(2):
                        hl = 2 * pt + j
                        ps_av = avps.tile([D + 1, QB], f32, tag="av")
                        for skc in range(nsk):
                            nc.tensor.matmul(
                                ps_av[:], vp[:, skc, hl * (D + 1):(hl + 1) * (D + 1)],
                                et[j][:, skc, :],
                                start=(skc == 0), stop=(skc == nsk - 1))
                        nc.vector.tensor_copy(
                            out=o_un[64 * j:64 * j + 64, pt, q0:q0 + QB],
                            in_=ps_av[0:D, :])
                        nc.vector.tensor_copy(
                            out=sums[j][0:1, q0:q0 + QB], in_=ps_av[D:D + 1, :])
                    # trailing qpT projection blocks interleave with attention
                    # pt=0: full-array matmuls keep the PE dense (and warm).
                    if pt == 0 and qb + 1 < S // QB:
                        proj_q(qb + 1)
                # normalize pair pt
                for j in range(2):
                    s128 = smpool.tile([128, S // 128], f32, tag="s128")
                    nc.sync.dma_start(s128[:], sums[j][0:1, :])
                    nc.vector.reciprocal(out=s128[:], in_=s128[:])
                    r128 = smpool.tile([128, S // 128], f32r, tag="r128")
                    nc.vector.tensor_copy(out=r128[:], in_=s128[:])
                    rc_r = rcpool.tile([1, S], f32r, tag="rcr")
                    nc.sync.dma_start(rc_r[0:1, :], r128[:])
                    for qb in range(S // 512):
                        rc_ps = avps.tile([64, 512], f32, tag="av", name="rc_ps")
                        nc.tensor.matmul(
                            rc_ps[:], ones_t[:], rc_r[0:1, qb * 512:(qb + 1) * 512],
                            start=True, stop=True)
                        nc.vector.tensor_tensor(
                            out=o_f16[64 * j:64 * j + 64, pt, qb * 512:(qb + 1) * 512],
                            in0=o_un[64 * j:64 * j + 64, pt, qb * 512:(qb + 1) * 512],
                            in1=rc_ps[:], op=Alu.mult)
                # fc_out pass for this pair
                for sqc in range(S // 128):
                    for eb in range(2):
                        ps = fps.tile([128, 512], f32, tag="fc")
                        nc.tensor.matmul(
                            ps[:], o_f16[:, pt, sqc * 128:(sqc + 1) * 128],
                            wo_t[:, pt, eb * 512:(eb + 1) * 512],
                            start=True, stop=True)
                        ob = opool.tile([128, 512], f16, tag="ob")
                        nc.any.tensor_copy(out=ob[:], in_=ps[:])
                        nc.sync.dma_start(
                            out_d[pt, sqc * 128:(sqc + 1) * 128,
                                  eb * 512:(eb + 1) * 512],
                            ob[:])

    if split_waits:
        _split_excess_waits(nc)
    return nc


def _prep_inputs(q, k, v, mask, W_qkv, b_qkv, W_out, b_out):
    """Host-side shard/layout prep. Returns (skv, in_maps)."""
    q = np.asarray(q, dtype=np.float32)
    k = np.asarray(k, dtype=np.float32)
    v = np.asarray(v, dtype=np.float32)
    mask = np.asarray(mask)
    W_qkv = np.asarray(W_qkv, dtype=np.float32)
    b_qkv = np.asarray(b_qkv, dtype=np.float32)
    W_out = np.asarray(W_out, dtype=np.float32)

    valid = [np.nonzero(mask[b, 0, 0] != 0)[0] for b in range(B)]
    cnts = [len(vi) for vi in valid]
    skv = max(128, max((c + 127) // 128 * 128 for c in cnts))

    # per-batch tensors
    qT, kTc, vTc, vms = [], [], [], []
    for b in range(B):
        qT.append(np.ascontiguousarray(q[b].T))
        kt = np.zeros((E, skv), np.float32)
        vt = np.zeros((E, skv), np.float16)
        kt[:, :cnts[b]] = k[b][valid[b]].T
        vt[:, :cnts[b]] = v[b][valid[b]].T
        kTc.append(kt)
        vTc.append(vt)
        vm = np.zeros((skv,), np.float32)
        vm[:cnts[b]] = 1.0
        vms.append(vm)

    in_maps = []
    for c in range(NCORES):
        b, g = divmod(c, GROUPS)
        sl = slice(g * DC, (g + 1) * DC)
        in_maps.append({
            "xqT": qT[b], "xkT": kTc[b], "xvT": vTc[b],
            "wqT": np.ascontiguousarray(W_qkv[sl, :].T),
            "wkT": np.ascontiguousarray(W_qkv[E:][sl, :].T),
            "wvT": np.ascontiguousarray(W_qkv[2 * E:][sl, :].T).astype(np.float16),
            "woT": np.ascontiguousarray(W_out[:, sl].T).astype(np.float16),
            "bq": np.ascontiguousarray(b_qkv[sl]),
            "bk": np.ascontiguousarray(b_qkv[E:][sl]),
            "bv": np.ascontiguousarray(b_qkv[2 * E:][sl]),
            "vmask": vms[b],
            "ones64": np.ones((1, 64), np.float32),
        })
    return skv, in_maps


def kernel(q, k, v, mask, W_qkv, b_qkv, W_out, b_out):
    from concourse import bass_utils

    skv, in_maps = _prep_inputs(q, k, v, mask, W_qkv, b_qkv, W_out, b_out)
    if skv not in _CACHE:
        _CACHE[skv] = _build(skv)
    nc = _CACHE[skv]

    trace = os.environ.get("KERNEL_TRACE") == "1"
    if trace:
        bass_utils.upload_artifacts = lambda tmpdir: "local://" + tmpdir
    res = bass_utils.run_bass_kernel_spmd(
        nc, in_maps, list(range(NCORES)), trace=trace)
    if trace:
        print(f"HW exec time: {res.exec_time_ns} ns")
        if res.instructions_and_trace is not None:
            print(f"trace path: {res.instructions_and_trace[1]}")

    b_out = np.asarray(b_out, dtype=np.float32)
    out = np.zeros((B, S, E), np.float32)
    for c in range(NCORES):
        out[c // GROUPS] += res.results[c]["out"].astype(np.float32).sum(axis=0)
    out += b_out[None, None, :]
    return out



# revision 17
# speedup vs baseline: 1.0933x; 1.0933x over previous
"""Multi-head attention (B=2, S=2048, E=1024, H=16) on 8 TRN2 NeuronCores.

Sharding: batch x head-group. Core c handles batch b=c//4 and head group
g=c%4 (4 heads = 256 of E). Each core computes its heads' attention output
slice and a partial fc_out product [S, E]; the host sums the 4 partials per
batch and adds b_out.

v2 design notes (vs the 239us baseline):
- All HBM input tensors are f16 (host-cast); qpT/kpT kept on-chip in f32r.
- Single fused [S, E] f16 output per core: fc_out contracts K=256 over both
  head-pairs (2 accumulating matmuls) -> half the output DMA and copies.
- Loop order: outer qb (512-query block), inner pt (head pair). Normalize
  and fc_out are software-pipelined one unit behind attention so the PE
  instruction stream never waits on the recip chain (keeps PE HAM-warm).
- Act engine runs ONLY exp, as [128, 2, 512] pair-instructions spanning the
  two score psum banks. Copies are on gpsimd, recip/normalize-mult on DVE.
- Softmax denominator: ones-column trick in the AV matmul (row 64 of the
  [65, 512] psum); per-query reciprocal row is partition-broadcast with a
  single K=2 matmul against a constant [2, 128] "eye64" block matrix.
- K/V projections are chunked at 128 keys to pace with their DMAs, so the
  PE starts working ~4us in and stays continuously busy.

Mask handling is exact: masked K/V rows are removed on the host (gather),
so softmax(where(mask==0, -1e20, e)) == exp(e_valid)/sum(exp(e_valid)).
"""

import os

import numpy as np

B, S, E, H = 2, 2048, 1024, 16
D = E // H           # 64
NCORES = 8
GROUPS = 4           # head groups per batch (cores per batch)
HPG = H // GROUPS    # 4 heads per core
DC = E // GROUPS     # 256 dims per core
NB = E // 128        # 8 contraction chunks over E
QB = 512             # query block width
NQB = S // QB        # 4

_CACHE = {}


def _split_excess_waits(nc, max_waits=1):
    """walrus rejects instructions carrying >1 sem wait; spread extras onto
    single-wait NoOps inserted before the instruction on the same engine."""
    import concourse.mybir as mybir

    n_split = 0
    for f in nc.m.functions:
        for bb in f.blocks:
            out, changed = [], False
            for ins in bb.instructions:
                si = ins.sync_info
                if si is not None and si.on_wait is not None and len(si.on_wait) > max_waits:
                    waits = list(si.on_wait)
                    for w in waits[:-max_waits]:
                        out.append(mybir.InstNoOp(
                            name=nc.get_next_instruction_name(),
                            engine=ins.engine, ins=[], outs=[],
                            sync_info=mybir.SyncInfo(on_wait=[w], on_update=[])))
                        n_split += 1
                    ins.sync_info = mybir.SyncInfo(
                        on_wait=waits[-max_waits:], on_update=list(si.on_update))
                    changed = True
                out.append(ins)
            if changed:
                bb.instructions = out
    return n_split


def _build(skv, split_waits=True):
    import concourse.bass as bass
    import concourse.mybir as mybir
    import concourse.tile as tile

    f32 = mybir.dt.float32
    f32r = mybir.dt.float32r
    f16 = mybir.dt.float16
    bf16 = mybir.dt.bfloat16
    Alu = mybir.AluOpType
    Act = mybir.ActivationFunctionType

    nsk = skv // 128

    nc = bass.Bass()
    xqT = nc.declare_dram_parameter("xqT", [E, S], f16, isOutput=False)
    xkT = nc.declare_dram_parameter("xkT", [E, skv], f16, isOutput=False)
    xvT = nc.declare_dram_parameter("xvT", [E, skv], f16, isOutput=False)
    wqT = nc.declare_dram_parameter("wqT", [E, DC], f16, isOutput=False)
    wkT = nc.declare_dram_parameter("wkT", [E, DC], f16, isOutput=False)
    wvT = nc.declare_dram_parameter("wvT", [E, DC], f16, isOutput=False)
    woT = nc.declare_dram_parameter("woT", [DC, E], f16, isOutput=False)
    bq_d = nc.declare_dram_parameter("bq", [DC], f32, isOutput=False)
    bk_d = nc.declare_dram_parameter("bk", [DC], f32, isOutput=False)
    bv_d = nc.declare_dram_parameter("bv", [DC], f32, isOutput=False)
    vm_d = nc.declare_dram_parameter("vmask", [skv], f32, isOutput=False)
    ones_d = nc.declare_dram_parameter("ones64", [1, 64], f32r, isOutput=False)
    out_d = nc.declare_dram_parameter("out", [S, E], f16, isOutput=True)

    xqT_r = xqT.rearrange("(ko p) s -> p ko s", p=128)
    xkT_r = xkT.rearrange("(ko p) s -> p ko s", p=128)
    xvT_r = xvT.rearrange("(ko p) s -> p ko s", p=128)

    with tile.TileContext(nc) as tc:
        with (
            tc.tile_pool(name="weights", bufs=1) as wpool,
            tc.tile_pool(name="consts", bufs=1) as cpool,
            tc.tile_pool(name="persist", bufs=1) as ppool,
            tc.tile_pool(name="xq_s", bufs=2) as xqpool,
            tc.tile_pool(name="xk_s", bufs=2) as xkpool,
            tc.tile_pool(name="xv_s", bufs=2) as xvpool,
            tc.tile_pool(name="et", bufs=2) as etpool,
            tc.tile_pool(name="oun", bufs=3) as oupool,
            tc.tile_pool(name="rc2", bufs=3) as rcpool,
            tc.tile_pool(name="of16", bufs=2) as ofpool,
            tc.tile_pool(name="ob", bufs=3) as obpool,
            tc.tile_pool(name="sc_ps", bufs=2, space="PSUM") as aps,
            tc.tile_pool(name="av_ps", bufs=2, space="PSUM") as avps,
            tc.tile_pool(name="work_ps", bufs=2, space="PSUM") as wps,
        ):
            # ---- constants + weights (DMA order = urgency order) ----
            bq_t = cpool.tile([128, 2], f32, tag="bq")
            bk_t = cpool.tile([128, 2], f32, tag="bk")
            bv_t = cpool.tile([128, DC], f32, tag="bv")
            vm_t = cpool.tile([128, nsk], f32, tag="vm")
            ones_t = cpool.tile([1, 64], f32r, tag="ones")
            nc.sync.dma_start(bq_t[:], bq_d.rearrange("(c p) -> p c", p=128))
            nc.sync.dma_start(bk_t[:], bk_d.rearrange("(c p) -> p c", p=128))
            nc.sync.dma_start(bv_t[:], bv_d[None, :].to_broadcast((128, DC)))
            nc.sync.dma_start(vm_t[:], vm_d.rearrange("(s p) -> p s", p=128))
            nc.sync.dma_start(ones_t[:], ones_d[:])

            wq_t = wpool.tile([128, NB, DC], f16, tag="wq")
            wk_t = wpool.tile([128, NB, DC], f16, tag="wk")
            wv_t = wpool.tile([128, NB, DC], f16, tag="wv")
            wo_t = wpool.tile([128, DC // 128, E], f16, tag="wo")

            qpT = ppool.tile([128, 2, S], f32r, tag="qpT")
            kpT = ppool.tile([128, 2, skv], f32r, tag="kpT")
            vp = ppool.tile([128, nsk, HPG * (D + 1)], bf16, tag="vp")

            def proj_q(nb):
                xq = xqpool.tile([128, NB, QB], f16, tag="xq", name="xq")
                nc.sync.dma_start(xq[:], xqT_r[:, :, nb * QB:(nb + 1) * QB])
                for mc in range(2):
                    ps = wps.tile([128, QB], f32, tag="wp", name="qp_ps")
                    for kc in range(NB):
                        nc.tensor.matmul(
                            ps[:], wq_t[:, kc, mc * 128:(mc + 1) * 128],
                            xq[:, kc, :], start=(kc == 0), stop=(kc == NB - 1))
                    nc.vector.tensor_tensor(
                        out=qpT[:, mc, nb * QB:(nb + 1) * QB], in0=ps[:],
                        in1=bq_t[:, mc:mc + 1].to_broadcast((128, QB)), op=Alu.add)

            def proj_k(sc):
                xk = xkpool.tile([128, NB, 128], f16, tag="xk", name="xk")
                nc.sync.dma_start(xk[:], xkT_r[:, :, sc * 128:(sc + 1) * 128])
                for mc in range(2):
                    ps = wps.tile([128, QB], f32, tag="wp", name="kp_ps")[:, :128]
                    for kc in range(NB):
                        nc.tensor.matmul(
                            ps[:], wk_t[:, kc, mc * 128:(mc + 1) * 128],
                            xk[:, kc, :], start=(kc == 0), stop=(kc == NB - 1))
                    nc.vector.tensor_tensor(
                        out=kpT[:, mc, sc * 128:(sc + 1) * 128], in0=ps[:],
                        in1=bk_t[:, mc:mc + 1].to_broadcast((128, 128)), op=Alu.add)

            def proj_v(sc):
                xv = xvpool.tile([128, NB, 128], f16, tag="xv", name="xv")
                nc.sync.dma_start(xv[:], xvT_r[:, :, sc * 128:(sc + 1) * 128])
                ps = wps.tile([128, QB], f32, tag="wp", name="vp_ps")[:, :DC]
                for kc in range(NB):
                    nc.tensor.matmul(
                        ps[:], xv[:, kc, :], wv_t[:, kc, :],
                        start=(kc == 0), stop=(kc == NB - 1))
                t1 = oupool.tile([128, DC], f32, tag="vtmp", name="vtmp")
                nc.vector.tensor_tensor(out=t1[:], in0=ps[:], in1=bv_t[:], op=Alu.add)
                vps = vp[:, sc, :].rearrange("p (h w) -> p h w", w=D + 1)
                nc.gpsimd.tensor_tensor(
                    out=vps[:, :, 0:D],
                    in0=t1.rearrange("p (h w) -> p h w", w=D),
                    in1=vm_t[:, sc:sc + 1, None].to_broadcast((128, HPG, D)),
                    op=Alu.mult)
                nc.gpsimd.tensor_copy(
                    out=vps[:, :, D:D + 1],
                    in_=vm_t[:, sc:sc + 1, None].to_broadcast((128, HPG, 1)))

            # ---- lead-in: Q block 0, then chunked K and V projections ----
            nc.sync.dma_start(wq_t[:], wqT.rearrange("(ko p) m -> p ko m", p=128))
            proj_q(0)
            nc.sync.dma_start(wk_t[:], wkT.rearrange("(ko p) m -> p ko m", p=128))
            for sc in range(nsk):
                proj_k(sc)
            nc.sync.dma_start(wv_t[:], wvT.rearrange("(ko p) m -> p ko m", p=128))
            for sc in range(nsk):
                proj_v(sc)
            nc.sync.dma_start(wo_t[:], woT.rearrange("(ko p) n -> p ko n", p=128))

            # ---- main loop: outer qb, inner pt; pipelined norm + fc ----
            of16 = {}
            pending_norm = []   # (qb, pt, o_unp, rc2t)

            def flush_norm():
                while pending_norm:
                    qb, pt, o_unp, rcs = pending_norm.pop(0)
                    if qb not in of16:
                        of16[qb] = ofpool.tile(
                            [128, 2, QB], f16, tag="of", name=f"of16_{qb}")
                    for j in range(2):
                        rc_ps = avps.tile([64, QB], f32, tag="av", name="rc_ps")
                        nc.tensor.matmul(
                            rc_ps[:], ones_t[:], rcs[j][:],
                            start=True, stop=True)
                        nc.vector.tensor_tensor(
                            out=of16[qb][64 * j:64 * j + 64, pt, :],
                            in0=o_unp[64 * j:64 * j + 64, :], in1=rc_ps[:],
                            op=Alu.mult)

            def flush_fc(qb):
                o = of16.pop(qb)
                for sqc in range(QB // 128):
                    ob = obpool.tile([128, 2, QB], f16, tag="ob", name="ob")
                    for eb in range(2):
                        fps = wps.tile([128, QB], f32, tag="wp", name="fc_ps")
                        nc.tensor.matmul(
                            fps[:], o[:, 0, sqc * 128:(sqc + 1) * 128],
                            wo_t[:, 0, eb * QB:(eb + 1) * QB],
                            start=True, stop=False)
                        nc.tensor.matmul(
                            fps[:], o[:, 1, sqc * 128:(sqc + 1) * 128],
                            wo_t[:, 1, eb * QB:(eb + 1) * QB],
                            start=False, stop=True)
                        nc.vector.tensor_copy(out=ob[:, eb, :], in_=fps[:])
                    nc.sync.dma_start(
                        out_d[qb * QB + sqc * 128: qb * QB + (sqc + 1) * 128, :],
                        ob[:])

            for qb in range(NQB):
                for pt in range(2):
                    et = etpool.tile([128, nsk, 2, QB], bf16, tag="et", name="et")
                    for skc in range(nsk):
                        psx = aps.tile([128, 2, QB], f32, tag="sc", name="psx")
                        for j in range(2):
                            nc.tensor.matmul(
                                psx[:, j, :],
                                kpT[64 * j:64 * j + 64, pt, skc * 128:(skc + 1) * 128],
                                qpT[64 * j:64 * j + 64, pt, qb * QB:(qb + 1) * QB],
                                start=True, stop=True, tile_position=(64 * j, 0))
                        nc.scalar.activation(et[:, skc, :, :], psx[:], Act.Exp)
                    o_unp = oupool.tile([128, QB], f32, tag="ou", name="o_unp")
                    rcs = [rcpool.tile([1, QB], f32r, tag=f"rc{j}", name=f"rc{j}")
                           for j in range(2)]
                    for j in range(2):
                        hl = 2 * pt + j
                        ps_av = avps.tile([D + 1, QB], f32, tag="av", name="ps_av")
                        for skc in range(nsk):
                            nc.tensor.matmul(
                                ps_av[:], vp[:, skc, hl * (D + 1):(hl + 1) * (D + 1)],
                                et[:, skc, j, :],
                                start=(skc == 0), stop=(skc == nsk - 1))
                        nc.vector.tensor_copy(
                            out=o_unp[64 * j:64 * j + 64, :], in_=ps_av[0:D, :])
                        with nc.allow_low_precision(
                                reason="softmax denom recip as f32r matmul rhs"):
                            nc.vector.reciprocal(
                                out=rcs[j][:], in_=ps_av[D:D + 1, :])
                    flush_norm()
                    pending_norm.append((qb, pt, o_unp, rcs))
                    if pt == 0 and qb + 1 < NQB:
                        proj_q(qb + 1)
                    if pt == 1 and qb >= 1:
                        flush_fc(qb - 1)
            flush_norm()
            flush_fc(NQB - 1)

    if split_waits:
        _split_excess_waits(nc)
    return nc


def _prep_inputs(q, k, v, mask, W_qkv, b_qkv, W_out, b_out):
    """Host-side shard/layout prep. Returns (skv, in_maps)."""
    q = np.asarray(q, dtype=np.float32)
    k = np.asarray(k, dtype=np.float32)
    v = np.asarray(v, dtype=np.float32)
    mask = np.asarray(mask)
    W_qkv = np.asarray(W_qkv, dtype=np.float32)
    b_qkv = np.asarray(b_qkv, dtype=np.float32)
    W_out = np.asarray(W_out, dtype=np.float32)

    valid = [np.nonzero(mask[b, 0, 0] != 0)[0] for b in range(B)]
    cnts = [len(vi) for vi in valid]
    skv = max(128, max((c + 127) // 128 * 128 for c in cnts))

    # per-batch tensors
    qT, kTc, vTc, vms = [], [], [], []
    for b in range(B):
        qT.append(np.ascontiguousarray(q[b].T).astype(np.float16))
        kt = np.zeros((E, skv), np.float16)
        vt = np.zeros((E, skv), np.float16)
        kt[:, :cnts[b]] = k[b][valid[b]].T
        vt[:, :cnts[b]] = v[b][valid[b]].T
        kTc.append(kt)
        vTc.append(vt)
        vm = np.zeros((skv,), np.float32)
        vm[:cnts[b]] = 1.0
        vms.append(vm)

    in_maps = []
    for c in range(NCORES):
        b, g = divmod(c, GROUPS)
        sl = slice(g * DC, (g + 1) * DC)
        in_maps.append({
            "xqT": qT[b], "xkT": kTc[b], "xvT": vTc[b],
            "wqT": np.ascontiguousarray(W_qkv[sl, :].T).astype(np.float16),
            "wkT": np.ascontiguousarray(W_qkv[E:][sl, :].T).astype(np.float16),
            "wvT": np.ascontiguousarray(W_qkv[2 * E:][sl, :].T).astype(np.float16),
            "woT": np.ascontiguousarray(W_out[:, sl].T).astype(np.float16),
            "bq": np.ascontiguousarray(b_qkv[sl]),
            "bk": np.ascontiguousarray(b_qkv[E:][sl]),
            "bv": np.ascontiguousarray(b_qkv[2 * E:][sl]),
            "vmask": vms[b],
            "ones64": np.ones((1, 64), np.float32),
        })
    return skv, in_maps


def kernel(q, k, v, mask, W_qkv, b_qkv, W_out, b_out):
    from concourse import bass_utils

    skv, in_maps = _prep_inputs(q, k, v, mask, W_qkv, b_qkv, W_out, b_out)
    if skv not in _CACHE:
        _CACHE[skv] = _build(skv)
    nc = _CACHE[skv]

    trace = os.environ.get("KERNEL_TRACE") == "1"
    if trace:
        bass_utils.upload_artifacts = lambda tmpdir: "local://" + tmpdir
    res = bass_utils.run_bass_kernel_spmd(
        nc, in_maps, list(range(NCORES)), trace=trace)
    if trace:
        print(f"HW exec time: {res.exec_time_ns} ns")
        if res.instructions_and_trace is not None:
            print(f"trace path: {res.instructions_and_trace[1]}")

    b_out = np.asarray(b_out, dtype=np.float32)
    out = np.zeros((B, S, E), np.float32)
    for c in range(NCORES):
        out[c // GROUPS] += res.results[c]["out"].astype(np.float32)
    out += b_out[None, None, :]
    return out


# revision 22
# speedup vs baseline: 1.2412x; 1.1353x over previous
"""Multi-head attention (B=2, S=2048, E=1024, H=16) on 8 TRN2 NeuronCores.

Sharding: batch x head-group. Core c handles batch b=c//4 and head group
g=c%4 (4 heads = 256 of E). Each core computes its heads' attention output
slice and a partial fc_out product [S, E]; the host sums the 4 partials per
batch and adds b_out.

v2 design notes (vs the 239us baseline):
- All HBM input tensors are f16 (host-cast); qpT/kpT kept on-chip in f32r.
- Single fused [S, E] f16 output per core: fc_out contracts K=256 over both
  head-pairs (2 accumulating matmuls) -> half the output DMA and copies.
- Loop order: outer qb (512-query block), inner pt (head pair). Normalize
  and fc_out are software-pipelined one unit behind attention so the PE
  instruction stream never waits on the recip chain (keeps PE HAM-warm).
- Act engine runs ONLY exp, as [128, 2, 512] pair-instructions spanning the
  two score psum banks. Copies are on gpsimd, recip/normalize-mult on DVE.
- Softmax denominator: ones-column trick in the AV matmul (row 64 of the
  [65, 512] psum); per-query reciprocal row is partition-broadcast with a
  single K=2 matmul against a constant [2, 128] "eye64" block matrix.
- K/V projections are chunked at 128 keys to pace with their DMAs, so the
  PE starts working ~4us in and stays continuously busy.

Mask handling is exact: masked K/V rows are removed on the host (gather),
so softmax(where(mask==0, -1e20, e)) == exp(e_valid)/sum(exp(e_valid)).
"""

import os

import numpy as np

B, S, E, H = 2, 2048, 1024, 16
D = E // H           # 64
NCORES = 8
GROUPS = 4           # head groups per batch (cores per batch)
HPG = H // GROUPS    # 4 heads per core
DC = E // GROUPS     # 256 dims per core
NB = E // 128        # 8 contraction chunks over E
QB = 512             # query block width
NQB = S // QB        # 4

_CACHE = {}


def _split_excess_waits(nc, max_waits=1):
    """walrus rejects instructions carrying >1 sem wait; spread extras onto
    single-wait NoOps inserted before the instruction on the same engine."""
    import concourse.mybir as mybir

    n_split = 0
    for f in nc.m.functions:
        for bb in f.blocks:
            out, changed = [], False
            for ins in bb.instructions:
                si = ins.sync_info
                if si is not None and si.on_wait is not None and len(si.on_wait) > max_waits:
                    waits = list(si.on_wait)
                    for w in waits[:-max_waits]:
                        out.append(mybir.InstNoOp(
                            name=nc.get_next_instruction_name(),
                            engine=ins.engine, ins=[], outs=[],
                            sync_info=mybir.SyncInfo(on_wait=[w], on_update=[])))
                        n_split += 1
                    ins.sync_info = mybir.SyncInfo(
                        on_wait=waits[-max_waits:], on_update=list(si.on_update))
                    changed = True
                out.append(ins)
            if changed:
                bb.instructions = out
    return n_split


def _build(skv, split_waits=True):
    import concourse.bass as bass
    import concourse.mybir as mybir
    import concourse.tile as tile

    f32 = mybir.dt.float32
    f32r = mybir.dt.float32r
    f16 = mybir.dt.float16
    bf16 = mybir.dt.bfloat16
    Alu = mybir.AluOpType
    Act = mybir.ActivationFunctionType

    nsk = skv // 128

    nc = bass.Bass()
    xqT = nc.declare_dram_parameter("xqT", [E, S], f16, isOutput=False)
    xkT = nc.declare_dram_parameter("xkT", [E, skv], f16, isOutput=False)
    xvT = nc.declare_dram_parameter("xvT", [E, skv], f16, isOutput=False)
    wqT = nc.declare_dram_parameter("wqT", [E, DC], f16, isOutput=False)
    wkT = nc.declare_dram_parameter("wkT", [E, DC], f16, isOutput=False)
    wvT = nc.declare_dram_parameter("wvT", [E, DC], f16, isOutput=False)
    woT = nc.declare_dram_parameter("woT", [DC, E], f16, isOutput=False)
    bq_d = nc.declare_dram_parameter("bq", [DC], f32, isOutput=False)
    bk_d = nc.declare_dram_parameter("bk", [DC], f32, isOutput=False)
    bv_d = nc.declare_dram_parameter("bv", [DC], f32, isOutput=False)
    vm_d = nc.declare_dram_parameter("vmask", [skv], f32, isOutput=False)
    ones_d = nc.declare_dram_parameter("ones64", [1, 64], f32r, isOutput=False)
    out_d = nc.declare_dram_parameter("out", [S, E], f16, isOutput=True)

    xqT_r = xqT.rearrange("(ko p) s -> p ko s", p=128)
    xkT_r = xkT.rearrange("(ko p) s -> p ko s", p=128)
    xvT_r = xvT.rearrange("(ko p) s -> p ko s", p=128)

    with tile.TileContext(nc) as tc:
        with (
            tc.tile_pool(name="weights", bufs=1) as wpool,
            tc.tile_pool(name="consts", bufs=1) as cpool,
            tc.tile_pool(name="persist", bufs=1) as ppool,
            tc.tile_pool(name="xq_s", bufs=2) as xqpool,
            tc.tile_pool(name="xk_s", bufs=2) as xkpool,
            tc.tile_pool(name="xv_s", bufs=2) as xvpool,
            tc.tile_pool(name="et", bufs=2) as etpool,
            tc.tile_pool(name="oun", bufs=3) as oupool,
            tc.tile_pool(name="rc2", bufs=3) as rcpool,
            tc.tile_pool(name="of16", bufs=2) as ofpool,
            tc.tile_pool(name="ob", bufs=3) as obpool,
            tc.tile_pool(name="sc_ps", bufs=2, space="PSUM") as aps,
            tc.tile_pool(name="av_ps", bufs=2, space="PSUM") as avps,
            tc.tile_pool(name="work_ps", bufs=2, space="PSUM") as wps,
        ):
            # ---- constants + weights (DMA order = urgency order) ----
            bq_t = cpool.tile([128, 2], f32, tag="bq")
            bk_t = cpool.tile([128, 2], f32, tag="bk")
            bv_t = cpool.tile([128, DC], f32, tag="bv")
            vm_t = cpool.tile([128, nsk], f32, tag="vm")
            ones_t = cpool.tile([1, 64], f32r, tag="ones")
            nc.sync.dma_start(bq_t[:], bq_d.rearrange("(c p) -> p c", p=128))
            nc.sync.dma_start(bk_t[:], bk_d.rearrange("(c p) -> p c", p=128))
            nc.sync.dma_start(bv_t[:], bv_d[None, :].to_broadcast((128, DC)))
            nc.sync.dma_start(vm_t[:], vm_d.rearrange("(s p) -> p s", p=128))
            nc.sync.dma_start(ones_t[:], ones_d[:])

            wq_t = wpool.tile([128, NB, DC], f16, tag="wq")
            wk_t = wpool.tile([128, NB, DC], f16, tag="wk")
            wv_t = wpool.tile([128, NB, DC], f16, tag="wv")
            wo_t = wpool.tile([128, DC // 128, E], f16, tag="wo")

            qpT = ppool.tile([128, 2, S], f32r, tag="qpT")
            kpT = ppool.tile([128, 2, skv], f32r, tag="kpT")
            vp = ppool.tile([128, nsk, HPG * (D + 1)], bf16, tag="vp")

            def proj_q(nb):
                xq = xqpool.tile([128, NB, QB], f16, tag="xq", name="xq")
                nc.sync.dma_start(xq[:], xqT_r[:, :, nb * QB:(nb + 1) * QB])
                for mc in range(2):
                    ps = wps.tile([128, QB], f32, tag="wp", name="qp_ps")
                    for kc in range(NB):
                        nc.tensor.matmul(
                            ps[:], wq_t[:, kc, mc * 128:(mc + 1) * 128],
                            xq[:, kc, :], start=(kc == 0), stop=(kc == NB - 1))
                    nc.vector.tensor_tensor(
                        out=qpT[:, mc, nb * QB:(nb + 1) * QB], in0=ps[:],
                        in1=bq_t[:, mc:mc + 1].to_broadcast((128, QB)), op=Alu.add)

            def proj_k(sc):
                xk = xkpool.tile([128, NB, 128], f16, tag="xk", name="xk")
                nc.sync.dma_start(xk[:], xkT_r[:, :, sc * 128:(sc + 1) * 128])
                for mc in range(2):
                    ps = wps.tile([128, QB], f32, tag="wp", name="kp_ps")[:, :128]
                    for kc in range(NB):
                        nc.tensor.matmul(
                            ps[:], wk_t[:, kc, mc * 128:(mc + 1) * 128],
                            xk[:, kc, :], start=(kc == 0), stop=(kc == NB - 1))
                    nc.vector.tensor_tensor(
                        out=kpT[:, mc, sc * 128:(sc + 1) * 128], in0=ps[:],
                        in1=bk_t[:, mc:mc + 1].to_broadcast((128, 128)), op=Alu.add)

            def proj_v(sc):
                xv = xvpool.tile([128, NB, 128], f16, tag="xv", name="xv")
                nc.sync.dma_start(xv[:], xvT_r[:, :, sc * 128:(sc + 1) * 128])
                ps = wps.tile([128, QB], f32, tag="wp", name="vp_ps")[:, :DC]
                for kc in range(NB):
                    nc.tensor.matmul(
                        ps[:], xv[:, kc, :], wv_t[:, kc, :],
                        start=(kc == 0), stop=(kc == NB - 1))
                t1 = oupool.tile([128, DC], f32, tag="vtmp", name="vtmp")
                nc.vector.tensor_tensor(out=t1[:], in0=ps[:], in1=bv_t[:], op=Alu.add)
                vps = vp[:, sc, :].rearrange("p (h w) -> p h w", w=D + 1)
                nc.gpsimd.tensor_tensor(
                    out=vps[:, :, 0:D],
                    in0=t1.rearrange("p (h w) -> p h w", w=D),
                    in1=vm_t[:, sc:sc + 1, None].to_broadcast((128, HPG, D)),
                    op=Alu.mult)
                nc.gpsimd.tensor_copy(
                    out=vps[:, :, D:D + 1],
                    in_=vm_t[:, sc:sc + 1, None].to_broadcast((128, HPG, 1)))

            # ---- lead-in: Q block 0, then chunked K projections ----
            nc.sync.dma_start(wq_t[:], wqT.rearrange("(ko p) m -> p ko m", p=128))
            proj_q(0)
            nc.sync.dma_start(wk_t[:], wkT.rearrange("(ko p) m -> p ko m", p=128))
            for sc in range(nsk):
                proj_k(sc)
            nc.sync.dma_start(wv_t[:], wvT.rearrange("(ko p) m -> p ko m", p=128))

            # ---- main loop ----
            # Software pipeline (unit u = (qb, pt)): scores(u) stream to the
            # Act engine while AV(u-1) matmuls interleave between them at skc
            # granularity, so the PE never parks at an AV waiting for exp(u)
            # and the Act engine never starves. Normalize runs at u+2, fc(qb)
            # once both pairs' normalize are out.
            of16 = {}
            ets = {}
            pending_norm = []   # (qb, pt, o_unp, rcs)
            pending_av = []     # (qb, pt)

            def flush_norm():
                while pending_norm:
                    qb, pt, o_unp, rc2s = pending_norm.pop(0)
                    if qb not in of16:
                        of16[qb] = ofpool.tile(
                            [128, 2, QB], f16, tag="of", name=f"of16_{qb}")
                    for j in range(2):
                        rc_ps = avps.tile([64, QB], f32, tag="av", name="rc_ps")
                        nc.tensor.matmul(
                            rc_ps[:], ones_t[:], rc2s[0:1, j, :],
                            start=True, stop=True)
                        nc.vector.tensor_tensor(
                            out=of16[qb][64 * j:64 * j + 64, pt, :],
                            in0=o_unp[64 * j:64 * j + 64, :], in1=rc_ps[:],
                            op=Alu.mult)

            def flush_fc(qb):
                o = of16.pop(qb)
                for sqc in range(QB // 128):
                    ob = obpool.tile([128, 2, QB], f16, tag="ob", name="ob")
                    for eb in range(2):
                        fps = wps.tile([128, QB], f32, tag="wp", name="fc_ps")
                        nc.tensor.matmul(
                            fps[:], o[:, 0, sqc * 128:(sqc + 1) * 128],
                            wo_t[:, 0, eb * QB:(eb + 1) * QB],
                            start=True, stop=False)
                        nc.tensor.matmul(
                            fps[:], o[:, 1, sqc * 128:(sqc + 1) * 128],
                            wo_t[:, 1, eb * QB:(eb + 1) * QB],
                            start=False, stop=True)
                        nc.vector.tensor_copy(out=ob[:, eb, :], in_=fps[:])
                    nc.sync.dma_start(
                        out_d[qb * QB + sqc * 128: qb * QB + (sqc + 1) * 128, :],
                        ob[:])

            def av_finish(qb, pt, ps_avs):
                """Drain one unit's AV psums: copy dims to SBUF; reciprocal of
                the two sums rows via a partition-packed [128, 8] round trip
                (plain [1, 512] reciprocal costs ~6.5ns/elem = 3.4us)."""
                o_unp = oupool.tile([128, QB], f32, tag="ou", name="o_unp")
                sums2 = rcpool.tile([1, 2, QB], f32, tag="sums", name="sums2")
                for j in range(2):
                    nc.vector.tensor_copy(
                        out=o_unp[64 * j:64 * j + 64, :], in_=ps_avs[j][0:D, :])
                    nc.vector.tensor_copy(
                        out=sums2[0:1, j, :], in_=ps_avs[j][D:D + 1, :])
                rcT = rcpool.tile([128, 2 * QB // 128], f32, tag="rcT", name="rcT")
                nc.sync.dma_start(rcT[:], sums2[0:1, :, :])
                rcT2 = rcpool.tile([128, 2 * QB // 128], f32r, tag="rcT2",
                                   name="rcT2")
                with nc.allow_low_precision(
                        reason="softmax denom recip as f32r matmul rhs"):
                    nc.vector.reciprocal(out=rcT2[:], in_=rcT[:])
                rc2s = rcpool.tile([1, 2, QB], f32r, tag="rc2s", name="rc2s")
                nc.sync.dma_start(rc2s[0:1, :, :], rcT2[:])
                pending_norm.append((qb, pt, o_unp, rc2s))

            units = [(qb, pt) for qb in range(NQB) for pt in range(2)]
            for ui, (qb, pt) in enumerate(units):
                flush_norm()   # normalize for unit u-2 (chain long since done)
                et = etpool.tile([128, nsk, 2, QB], bf16, tag="et", name="et")
                ets[(qb, pt)] = et
                prev = pending_av.pop(0) if pending_av else None
                if prev is not None:
                    pqb, ppt = prev
                    pet = ets.pop(prev)
                    ps_avs = [avps.tile([D + 1, QB], f32, tag="av",
                                        name=f"ps_av{j}") for j in range(2)]
                for skc in range(nsk):
                    psx = aps.tile([128, 2, QB], f32, tag="sc", name="psx")
                    for j in range(2):
                        nc.tensor.matmul(
                            psx[:, j, :],
                            kpT[64 * j:64 * j + 64, pt, skc * 128:(skc + 1) * 128],
                            qpT[64 * j:64 * j + 64, pt, qb * QB:(qb + 1) * QB],
                            start=True, stop=True, tile_position=(64 * j, 0))
                    nc.scalar.activation(et[:, skc, :, :], psx[:], Act.Exp)
                    if prev is not None:
                        for j in range(2):
                            hl = 2 * ppt + j
                            nc.tensor.matmul(
                                ps_avs[j][:],
                                vp[:, skc, hl * (D + 1):(hl + 1) * (D + 1)],
                                pet[:, skc, j, :],
                                start=(skc == 0), stop=(skc == nsk - 1),
                                skip_group_check=True)
                if prev is not None:
                    av_finish(pqb, ppt, ps_avs)
                pending_av.append((qb, pt))
                if ui == 0:
                    # exp(u0) wait window: fill the PE with the V projection
                    for sc in range(nsk):
                        proj_v(sc)
                    nc.sync.dma_start(
                        wo_t[:], woT.rearrange("(ko p) n -> p ko n", p=128))
                if pt == 0 and qb + 1 < NQB:
                    proj_q(qb + 1)
                if pt == 1 and qb >= 1:
                    flush_fc(qb - 1)
            # drain: AV for the last unit, then its normalize + final fc
            lqb, lpt = pending_av.pop(0)
            pet = ets.pop((lqb, lpt))
            ps_avs = [avps.tile([D + 1, QB], f32, tag="av", name=f"ps_av{j}")
                      for j in range(2)]
            for skc in range(nsk):
                for j in range(2):
                    hl = 2 * lpt + j
                    nc.tensor.matmul(
                        ps_avs[j][:], vp[:, skc, hl * (D + 1):(hl + 1) * (D + 1)],
                        pet[:, skc, j, :],
                        start=(skc == 0), stop=(skc == nsk - 1),
                        skip_group_check=True)
            av_finish(lqb, lpt, ps_avs)
            flush_norm()
            flush_fc(NQB - 1)

    if split_waits:
        _split_excess_waits(nc)
    return nc


def _prep_inputs(q, k, v, mask, W_qkv, b_qkv, W_out, b_out):
    """Host-side shard/layout prep. Returns (skv, in_maps)."""
    q = np.asarray(q, dtype=np.float32)
    k = np.asarray(k, dtype=np.float32)
    v = np.asarray(v, dtype=np.float32)
    mask = np.asarray(mask)
    W_qkv = np.asarray(W_qkv, dtype=np.float32)
    b_qkv = np.asarray(b_qkv, dtype=np.float32)
    W_out = np.asarray(W_out, dtype=np.float32)

    valid = [np.nonzero(mask[b, 0, 0] != 0)[0] for b in range(B)]
    cnts = [len(vi) for vi in valid]
    skv = max(128, max((c + 127) // 128 * 128 for c in cnts))

    # per-batch tensors
    qT, kTc, vTc, vms = [], [], [], []
    for b in range(B):
        qT.append(np.ascontiguousarray(q[b].T).astype(np.float16))
        kt = np.zeros((E, skv), np.float16)
        vt = np.zeros((E, skv), np.float16)
        kt[:, :cnts[b]] = k[b][valid[b]].T
        vt[:, :cnts[b]] = v[b][valid[b]].T
        kTc.append(kt)
        vTc.append(vt)
        vm = np.zeros((skv,), np.float32)
        vm[:cnts[b]] = 1.0
        vms.append(vm)

    in_maps = []
    for c in range(NCORES):
        b, g = divmod(c, GROUPS)
        sl = slice(g * DC, (g + 1) * DC)
        in_maps.append({
            "xqT": qT[b], "xkT": kTc[b], "xvT": vTc[b],
            "wqT": np.ascontiguousarray(W_qkv[sl, :].T).astype(np.float16),
            "wkT": np.ascontiguousarray(W_qkv[E:][sl, :].T).astype(np.float16),
            "wvT": np.ascontiguousarray(W_qkv[2 * E:][sl, :].T).astype(np.float16),
            "woT": np.ascontiguousarray(W_out[:, sl].T).astype(np.float16),
            "bq": np.ascontiguousarray(b_qkv[sl]),
            "bk": np.ascontiguousarray(b_qkv[E:][sl]),
            "bv": np.ascontiguousarray(b_qkv[2 * E:][sl]),
            "vmask": vms[b],
            "ones64": np.ones((1, 64), np.float32),
        })
    return skv, in_maps


def kernel(q, k, v, mask, W_qkv, b_qkv, W_out, b_out):
    from concourse import bass_utils

    skv, in_maps = _prep_inputs(q, k, v, mask, W_qkv, b_qkv, W_out, b_out)
    if skv not in _CACHE:
        _CACHE[skv] = _build(skv)
    nc = _CACHE[skv]

    trace = os.environ.get("KERNEL_TRACE") == "1"
    if trace:
        bass_utils.upload_artifacts = lambda tmpdir: "local://" + tmpdir
    res = bass_utils.run_bass_kernel_spmd(
        nc, in_maps, list(range(NCORES)), trace=trace)
    if trace:
        print(f"HW exec time: {res.exec_time_ns} ns")
        if res.instructions_and_trace is not None:
            print(f"trace path: {res.instructions_and_trace[1]}")

    b_out = np.asarray(b_out, dtype=np.float32)
    out = np.zeros((B, S, E), np.float32)
    for c in range(NCORES):
        out[c // GROUPS] += res.results[c]["out"].astype(np.float32)
    out += b_out[None, None, :]
    return out


# revision 30
# speedup vs baseline: 1.4320x; 1.1537x over previous
"""Multi-head attention (B=2, S=2048, E=1024, H=16) on 8 TRN2 NeuronCores.

Sharding: batch x head-group. Core c handles batch b=c//4 and head group
g=c%4 (4 heads = 256 of E). Each core computes its heads' attention output
slice and a partial fc_out product [S, E]; the host sums the 4 partials per
batch and adds b_out.

v2 design notes (vs the 239us baseline):
- All HBM input tensors are f16 (host-cast); qpT/kpT kept on-chip in f32r.
- Single fused [S, E] f16 output per core: fc_out contracts K=256 over both
  head-pairs (2 accumulating matmuls) -> half the output DMA and copies.
- Loop order: outer qb (512-query block), inner pt (head pair). Normalize
  and fc_out are software-pipelined one unit behind attention so the PE
  instruction stream never waits on the recip chain (keeps PE HAM-warm).
- Act engine runs ONLY exp, as [128, 2, 512] pair-instructions spanning the
  two score psum banks. Copies are on gpsimd, recip/normalize-mult on DVE.
- Softmax denominator: ones-column trick in the AV matmul (row 64 of the
  [65, 512] psum); per-query reciprocal row is partition-broadcast with a
  single K=2 matmul against a constant [2, 128] "eye64" block matrix.
- K/V projections are chunked at 128 keys to pace with their DMAs, so the
  PE starts working ~4us in and stays continuously busy.

Mask handling is exact: masked K/V rows are removed on the host (gather),
so softmax(where(mask==0, -1e20, e)) == exp(e_valid)/sum(exp(e_valid)).
"""

import os

import numpy as np

B, S, E, H = 2, 2048, 1024, 16
D = E // H           # 64
NCORES = 8
GROUPS = 4           # head groups per batch (cores per batch)
HPG = H // GROUPS    # 4 heads per core
DC = E // GROUPS     # 256 dims per core
NB = E // 128        # 8 contraction chunks over E
QB = 512             # query block width
NQB = S // QB        # 4

_CACHE = {}


def _split_excess_waits(nc, max_waits=1):
    """walrus rejects instructions carrying >1 sem wait; spread extras onto
    single-wait NoOps inserted before the instruction on the same engine."""
    import concourse.mybir as mybir

    n_split = 0
    for f in nc.m.functions:
        for bb in f.blocks:
            out, changed = [], False
            for ins in bb.instructions:
                si = ins.sync_info
                if si is not None and si.on_wait is not None and len(si.on_wait) > max_waits:
                    waits = list(si.on_wait)
                    for w in waits[:-max_waits]:
                        out.append(mybir.InstNoOp(
                            name=nc.get_next_instruction_name(),
                            engine=ins.engine, ins=[], outs=[],
                            sync_info=mybir.SyncInfo(on_wait=[w], on_update=[])))
                        n_split += 1
                    ins.sync_info = mybir.SyncInfo(
                        on_wait=waits[-max_waits:], on_update=list(si.on_update))
                    changed = True
                out.append(ins)
            if changed:
                bb.instructions = out
    return n_split


def _build(skv, split_waits=True):
    import concourse.bass as bass
    import concourse.mybir as mybir
    import concourse.tile as tile

    f32 = mybir.dt.float32
    f32r = mybir.dt.float32r
    f16 = mybir.dt.float16
    bf16 = mybir.dt.bfloat16
    Alu = mybir.AluOpType
    Act = mybir.ActivationFunctionType

    nsk = skv // 128

    nc = bass.Bass()
    xqT = nc.declare_dram_parameter("xqT", [E, S], f16, isOutput=False)
    xkT = nc.declare_dram_parameter("xkT", [E, skv], f16, isOutput=False)
    xvT = nc.declare_dram_parameter("xvT", [E, skv], f16, isOutput=False)
    wqT = nc.declare_dram_parameter("wqT", [E, DC], f16, isOutput=False)
    wkT = nc.declare_dram_parameter("wkT", [E, DC], f16, isOutput=False)
    wvT = nc.declare_dram_parameter("wvT", [E, DC], f16, isOutput=False)
    woT = nc.declare_dram_parameter("woT", [DC, E], f16, isOutput=False)
    bq_d = nc.declare_dram_parameter("bq", [DC], f32, isOutput=False)
    bk_d = nc.declare_dram_parameter("bk", [DC], f32, isOutput=False)
    bv_d = nc.declare_dram_parameter("bv", [DC], f32, isOutput=False)
    vm_d = nc.declare_dram_parameter("vmask", [skv], f32, isOutput=False)
    ones_d = nc.declare_dram_parameter("ones64", [1, 64], f32r, isOutput=False)
    out_d = nc.declare_dram_parameter("out", [S, E], f16, isOutput=True)

    xqT_r = xqT.rearrange("(ko p) s -> p ko s", p=128)
    xkT_r = xkT.rearrange("(ko p) s -> p ko s", p=128)
    xvT_r = xvT.rearrange("(ko p) s -> p ko s", p=128)

    with tile.TileContext(nc) as tc:
        with (
            tc.tile_pool(name="weights", bufs=1) as wpool,
            tc.tile_pool(name="consts", bufs=1) as cpool,
            tc.tile_pool(name="persist", bufs=1) as ppool,
            tc.tile_pool(name="xq_s", bufs=2) as xqpool,
            tc.tile_pool(name="xk_s", bufs=2) as xkpool,
            tc.tile_pool(name="xv_s", bufs=9) as xvpool,
            tc.tile_pool(name="et", bufs=2) as etpool,
            tc.tile_pool(name="oun", bufs=3) as oupool,
            tc.tile_pool(name="rc2", bufs=3) as rcpool,
            tc.tile_pool(name="of16", bufs=2) as ofpool,
            tc.tile_pool(name="ob", bufs=3) as obpool,
            tc.tile_pool(name="sc_ps", bufs=2, space="PSUM") as aps,
            tc.tile_pool(name="av_ps", bufs=2, space="PSUM") as avps,
            tc.tile_pool(name="work_ps", bufs=2, space="PSUM") as wps,
        ):
            # ---- constants + weights (DMA order = urgency order) ----
            bq_t = cpool.tile([128, 2], f32, tag="bq")
            bk_t = cpool.tile([128, 2], f32, tag="bk")
            bv_t = cpool.tile([128, DC], f32, tag="bv")
            vm_t = cpool.tile([128, nsk], f32, tag="vm")
            ones_t = cpool.tile([1, 64], f32r, tag="ones")

            wq_t = wpool.tile([128, NB, DC], f16, tag="wq")
            wk_t = wpool.tile([128, NB, DC], f16, tag="wk")
            wv_t = wpool.tile([128, NB, DC], f16, tag="wv")
            wo_t = wpool.tile([128, DC // 128, E], f16, tag="wo")

            qpT = ppool.tile([128, 2, S], f32r, tag="qpT")
            kpT = ppool.tile([128, 2, skv], f32r, tag="kpT")
            vp = ppool.tile([128, nsk, HPG * (D + 1)], bf16, tag="vp")

            def proj_q(nb):
                xq = xqpool.tile([128, NB, QB], f16, tag="xq", name="xq")
                nc.sync.dma_start(xq[:], xqT_r[:, :, nb * QB:(nb + 1) * QB])
                for mc in range(2):
                    ps = wps.tile([128, QB], f32, tag="wp", name="qp_ps")
                    for kc in range(NB):
                        nc.tensor.matmul(
                            ps[:], wq_t[:, kc, mc * 128:(mc + 1) * 128],
                            xq[:, kc, :], start=(kc == 0), stop=(kc == NB - 1))
                    nc.vector.tensor_tensor(
                        out=qpT[:, mc, nb * QB:(nb + 1) * QB], in0=ps[:],
                        in1=bq_t[:, mc:mc + 1].to_broadcast((128, QB)), op=Alu.add)

            def proj_k(sc):
                xk = xkpool.tile([128, NB, 128], f16, tag="xk", name="xk")
                nc.sync.dma_start(xk[:], xkT_r[:, :, sc * 128:(sc + 1) * 128])
                for mc in range(2):
                    ps = wps.tile([128, QB], f32, tag="wp", name="kp_ps")[:, :128]
                    for kc in range(NB):
                        nc.tensor.matmul(
                            ps[:], wk_t[:, kc, mc * 128:(mc + 1) * 128],
                            xk[:, kc, :], start=(kc == 0), stop=(kc == NB - 1))
                    nc.vector.tensor_tensor(
                        out=kpT[:, mc, sc * 128:(sc + 1) * 128], in0=ps[:],
                        in1=bk_t[:, mc:mc + 1].to_broadcast((128, 128)), op=Alu.add)

            xvs = []   # pre-issued xv chunk tiles (DMAs dispatched in lead-in)

            def proj_v(sc):
                xv = xvs[sc]
                ps = wps.tile([128, QB], f32, tag="wp", name="vp_ps")[:, :DC]
                for kc in range(NB):
                    nc.tensor.matmul(
                        ps[:], xv[:, kc, :], wv_t[:, kc, :],
                        start=(kc == 0), stop=(kc == NB - 1))
                t1 = oupool.tile([128, DC], f32, tag="vtmp", name="vtmp")
                nc.vector.tensor_tensor(out=t1[:], in0=ps[:], in1=bv_t[:], op=Alu.add)
                vps = vp[:, sc, :].rearrange("p (h w) -> p h w", w=D + 1)
                nc.gpsimd.tensor_tensor(
                    out=vps[:, :, 0:D],
                    in0=t1.rearrange("p (h w) -> p h w", w=D),
                    in1=vm_t[:, sc:sc + 1, None].to_broadcast((128, HPG, D)),
                    op=Alu.mult)
                nc.gpsimd.tensor_copy(
                    out=vps[:, :, D:D + 1],
                    in_=vm_t[:, sc:sc + 1, None].to_broadcast((128, HPG, 1)))

            # ---- lead-in ----
            # DMA dispatch is spread over SP (input streams) and Act (weights,
            # idle until the first exp) so the SP sequencer (565ns/trigger)
            # doesn't serialize the lead-in. SP order: xq0, consts, xk chunks,
            # xv chunks (prefetched into a 9-deep pool for the proj_v fillers).
            nc.scalar.dma_start(wq_t[:], wqT.rearrange("(ko p) m -> p ko m", p=128))
            nc.scalar.dma_start(wk_t[:], wkT.rearrange("(ko p) m -> p ko m", p=128))
            nc.scalar.dma_start(wv_t[:], wvT.rearrange("(ko p) m -> p ko m", p=128))
            nc.scalar.dma_start(wo_t[:], woT.rearrange("(ko p) n -> p ko n", p=128))
            nc.scalar.dma_start(bq_t[:], bq_d.rearrange("(c p) -> p c", p=128))
            nc.scalar.dma_start(bk_t[:], bk_d.rearrange("(c p) -> p c", p=128))
            nc.scalar.dma_start(bv_t[:], bv_d[None, :].to_broadcast((128, DC)))
            nc.scalar.dma_start(vm_t[:], vm_d.rearrange("(s p) -> p s", p=128))
            nc.scalar.dma_start(ones_t[:], ones_d[:])
            proj_q(0)
            for sc in range(nsk):
                proj_k(sc)
            for sc in range(nsk):
                xv = xvpool.tile([128, NB, 128], f16, tag="xv", name=f"xv{sc}")
                nc.sync.dma_start(xv[:], xvT_r[:, :, sc * 128:(sc + 1) * 128])
                xvs.append(xv)

            # ---- main loop ----
            # Software pipeline (unit u = (qb, pt)): scores(u) stream to the
            # Act engine while AV(u-1) matmuls interleave between them at skc
            # granularity, so the PE never parks at an AV waiting on exp(u)
            # and the Act engine never starves. All other PE work (normalize
            # broadcast, fc_out, next-block q projection) is queued as small
            # "filler" closures popped between steps; pops start at step 5 of
            # each unit so the normalize recip DMA round trip (~4us) has
            # settled before its broadcast matmul reaches the PE stream.
            of16 = {}
            ets = {}
            pending_av = []     # (qb, pt)
            filler_q = []

            def pop_filler(n=1):
                for _ in range(n):
                    if filler_q:
                        filler_q.pop(0)()

            def queue_norm(qb, pt, o_unp, rc2s):
                if qb not in of16:
                    of16[qb] = ofpool.tile(
                        [128, 2, QB], f16, tag="of", name=f"of16_{qb}")
                o = of16[qb]

                def mk(j):
                    def go():
                        rc_ps = avps.tile([64, QB], f32, tag="av", name="rc_ps")
                        nc.tensor.matmul(
                            rc_ps[:], ones_t[:], rc2s[0:1, j, :],
                            start=True, stop=True, skip_group_check=True)
                        nc.vector.tensor_tensor(
                            out=o[64 * j:64 * j + 64, pt, :],
                            in0=o_unp[64 * j:64 * j + 64, :], in1=rc_ps[:],
                            op=Alu.mult)
                    return go
                filler_q.append(mk(0))
                filler_q.append(mk(1))

            def queue_fc(qb):
                o = of16.pop(qb)
                obs = {}

                def mk(sqc, eb):
                    def go():
                        if sqc not in obs:
                            obs[sqc] = obpool.tile(
                                [128, 2, QB], f16, tag="ob", name="ob")
                        fps = wps.tile([128, QB], f32, tag="wp", name="fc_ps")
                        nc.tensor.matmul(
                            fps[:], o[:, 0, sqc * 128:(sqc + 1) * 128],
                            wo_t[:, 0, eb * QB:(eb + 1) * QB],
                            start=True, stop=False, skip_group_check=True)
                        nc.tensor.matmul(
                            fps[:], o[:, 1, sqc * 128:(sqc + 1) * 128],
                            wo_t[:, 1, eb * QB:(eb + 1) * QB],
                            start=False, stop=True, skip_group_check=True)
                        nc.vector.tensor_copy(out=obs[sqc][:, eb, :], in_=fps[:])
                        if eb == 1:
                            nc.sync.dma_start(
                                out_d[qb * QB + sqc * 128:
                                      qb * QB + (sqc + 1) * 128, :],
                                obs[sqc][:])
                    return go
                for sqc in range(QB // 128):
                    for eb in range(2):
                        filler_q.append(mk(sqc, eb))

            def queue_proj_q(nb):
                xq = xqpool.tile([128, NB, QB], f16, tag="xq", name="xq")
                nc.sync.dma_start(xq[:], xqT_r[:, :, nb * QB:(nb + 1) * QB])
                pss = {}

                def mk(mc):
                    def go():
                        ps = wps.tile([128, QB], f32, tag="wp", name="qp_ps")
                        pss[mc] = ps
                        for kc in range(NB):
                            nc.tensor.matmul(
                                ps[:], wq_t[:, kc, mc * 128:(mc + 1) * 128],
                                xq[:, kc, :], start=(kc == 0),
                                stop=(kc == NB - 1), skip_group_check=True)
                        nc.vector.tensor_tensor(
                            out=qpT[:, mc, nb * QB:(nb + 1) * QB], in0=ps[:],
                            in1=bq_t[:, mc:mc + 1].to_broadcast((128, QB)),
                            op=Alu.add)
                    return go
                filler_q.append(mk(0))
                filler_q.append(mk(1))

            def av_finish(qb, pt, ps_avs):
                """Drain one unit's AV psums: copy dims to SBUF; reciprocal of
                the two sums rows via a partition-packed [128, 8] round trip
                (plain [1, 512] reciprocal costs ~6.5ns/elem = 3.4us)."""
                o_unp = oupool.tile([128, QB], f32, tag="ou", name="o_unp")
                sums2 = rcpool.tile([1, 2, QB], f32, tag="sums", name="sums2")
                for j in range(2):
                    nc.vector.tensor_copy(
                        out=o_unp[64 * j:64 * j + 64, :], in_=ps_avs[j][0:D, :])
                    nc.vector.tensor_copy(
                        out=sums2[0:1, j, :], in_=ps_avs[j][D:D + 1, :])
                rcT = rcpool.tile([128, 2 * QB // 128], f32, tag="rcT", name="rcT")
                nc.sync.dma_start(rcT[:], sums2[0:1, :, :])
                rcT2 = rcpool.tile([128, 2 * QB // 128], f32r, tag="rcT2",
                                   name="rcT2")
                with nc.allow_low_precision(
                        reason="softmax denom recip as f32r matmul rhs"):
                    nc.vector.reciprocal(out=rcT2[:], in_=rcT[:])
                rc2s = rcpool.tile([1, 2, QB], f32r, tag="rc2s", name="rc2s")
                nc.sync.dma_start(rc2s[0:1, :, :], rcT2[:])
                queue_norm(qb, pt, o_unp, rc2s)

            def unit_steps(qb, pt, et, prev, pet, ps_avs, fill_from=5, fill_n=2):
                for skc in range(nsk):
                    if et is not None:
                        psx = aps.tile([128, 2, QB], f32, tag="sc", name="psx")
                        for j in range(2):
                            nc.tensor.matmul(
                                psx[:, j, :],
                                kpT[64 * j:64 * j + 64, pt,
                                    skc * 128:(skc + 1) * 128],
                                qpT[64 * j:64 * j + 64, pt,
                                    qb * QB:(qb + 1) * QB],
                                start=True, stop=True, tile_position=(64 * j, 0))
                        nc.scalar.activation(et[:, skc, :, :], psx[:], Act.Exp)
                    if prev is not None:
                        pqb, ppt = prev
                        for j in range(2):
                            hl = 2 * ppt + j
                            nc.tensor.matmul(
                                ps_avs[j][:],
                                vp[:, skc, hl * (D + 1):(hl + 1) * (D + 1)],
                                pet[:, skc, j, :],
                                start=(skc == 0), stop=(skc == nsk - 1),
                                skip_group_check=True)
                    if skc >= fill_from:
                        pop_filler(fill_n)

            # V projection runs as fillers inside unit u0's steps (its xv
            # DMAs are already in flight; AV(u0) only starts at unit u1).
            for sc in range(nsk):
                filler_q.append(lambda sc=sc: proj_v(sc))

            units = [(qb, pt) for qb in range(NQB) for pt in range(2)]
            for ui, (qb, pt) in enumerate(units):
                et = etpool.tile([128, nsk, 2, QB], bf16, tag="et", name="et")
                ets[(qb, pt)] = et
                prev = pending_av.pop(0) if pending_av else None
                pet = ps_avs = None
                if prev is not None:
                    pet = ets.pop(prev)
                    ps_avs = [avps.tile([D + 1, QB], f32, tag="av",
                                        name=f"ps_av{j}") for j in range(2)]
                if ui == 0:
                    unit_steps(qb, pt, et, prev, pet, ps_avs,
                               fill_from=0, fill_n=1)
                else:
                    unit_steps(qb, pt, et, prev, pet, ps_avs)
                if prev is not None:
                    av_finish(prev[0], prev[1], ps_avs)
                pending_av.append((qb, pt))
                if pt == 0 and qb + 1 < NQB:
                    queue_proj_q(qb + 1)
                if pt == 0 and qb >= 1:
                    queue_fc(qb - 1)
            # drain: AV for the last unit with fillers, then final norm + fc
            prev = pending_av.pop(0)
            pet = ets.pop(prev)
            ps_avs = [avps.tile([D + 1, QB], f32, tag="av", name=f"ps_av{j}")
                      for j in range(2)]
            unit_steps(None, None, None, prev, pet, ps_avs)
            av_finish(prev[0], prev[1], ps_avs)
            queue_fc(NQB - 1)
            pop_filler(len(filler_q))

    if split_waits:
        _split_excess_waits(nc)
    return nc


def _prep_inputs(q, k, v, mask, W_qkv, b_qkv, W_out, b_out):
    """Host-side shard/layout prep. Returns (skv, in_maps)."""
    q = np.asarray(q, dtype=np.float32)
    k = np.asarray(k, dtype=np.float32)
    v = np.asarray(v, dtype=np.float32)
    mask = np.asarray(mask)
    W_qkv = np.asarray(W_qkv, dtype=np.float32)
    b_qkv = np.asarray(b_qkv, dtype=np.float32)
    W_out = np.asarray(W_out, dtype=np.float32)

    valid = [np.nonzero(mask[b, 0, 0] != 0)[0] for b in range(B)]
    cnts = [len(vi) for vi in valid]
    skv = max(128, max((c + 127) // 128 * 128 for c in cnts))

    # per-batch tensors
    qT, kTc, vTc, vms = [], [], [], []
    for b in range(B):
        qT.append(np.ascontiguousarray(q[b].T).astype(np.float16))
        kt = np.zeros((E, skv), np.float16)
        vt = np.zeros((E, skv), np.float16)
        kt[:, :cnts[b]] = k[b][valid[b]].T
        vt[:, :cnts[b]] = v[b][valid[b]].T
        kTc.append(kt)
        vTc.append(vt)
        vm = np.zeros((skv,), np.float32)
        vm[:cnts[b]] = 1.0
        vms.append(vm)

    in_maps = []
    for c in range(NCORES):
        b, g = divmod(c, GROUPS)
        sl = slice(g * DC, (g + 1) * DC)
        in_maps.append({
            "xqT": qT[b], "xkT": kTc[b], "xvT": vTc[b],
            "wqT": np.ascontiguousarray(W_qkv[sl, :].T).astype(np.float16),
            "wkT": np.ascontiguousarray(W_qkv[E:][sl, :].T).astype(np.float16),
            "wvT": np.ascontiguousarray(W_qkv[2 * E:][sl, :].T).astype(np.float16),
            "woT": np.ascontiguousarray(W_out[:, sl].T).astype(np.float16),
            "bq": np.ascontiguousarray(b_qkv[sl]),
            "bk": np.ascontiguousarray(b_qkv[E:][sl]),
            "bv": np.ascontiguousarray(b_qkv[2 * E:][sl]),
            "vmask": vms[b],
            "ones64": np.ones((1, 64), np.float32),
        })
    return skv, in_maps


def kernel(q, k, v, mask, W_qkv, b_qkv, W_out, b_out):
    from concourse import bass_utils

    skv, in_maps = _prep_inputs(q, k, v, mask, W_qkv, b_qkv, W_out, b_out)
    if skv not in _CACHE:
        _CACHE[skv] = _build(skv)
    nc = _CACHE[skv]

    trace = os.environ.get("KERNEL_TRACE") == "1"
    if trace:
        bass_utils.upload_artifacts = lambda tmpdir: "local://" + tmpdir
    res = bass_utils.run_bass_kernel_spmd(
        nc, in_maps, list(range(NCORES)), trace=trace)
    if trace:
        print(f"HW exec time: {res.exec_time_ns} ns")
        if res.instructions_and_trace is not None:
            print(f"trace path: {res.instructions_and_trace[1]}")

    b_out = np.asarray(b_out, dtype=np.float32)
    out = np.zeros((B, S, E), np.float32)
    for c in range(NCORES):
        out[c // GROUPS] += res.results[c]["out"].astype(np.float32)
    out += b_out[None, None, :]
    return out
